# revision 13
# baseline (speedup 1.0000x reference)
"""Differential attention (B=2, T=2048, D=2048, H=16, HD=128) on 8 TRN2 cores.

Sharding: core c -> (batch b = c // 4, head-group g = c % 4); each core runs
batch b with 4 heads (4g..4g+3). Out-projection partials (over head groups)
are reduced on the host.

Per-core dataflow (one SPMD Bass program):
  inputs arrive as a handful of big host-pre-tiled DMAs (16KB/partition
  lines). Phase A projects t-block 0 only; t-blocks 1-3 and the
  out-projections are chopped into small "filler" pieces and pumped into
  the PE stream between attention steps so the PE never drains.

  softmax per (head, 128-row q-tile):
    scores ps1/ps2 via K=64 matmuls into [128,1024] PSUM spans; causal mask
    added by a PE matmul (I @ msk) accumulation; exp on ScalarE with fused
    row-sum accumulators; combine u = e1 + s*e2 with s = -lam*sum1/sum2 as a
    single scalar_tensor_tensor; normalization by 1/sum1 is folded into the
    PE transpose by using diag(1/sum1) as the transpose identity; attn @ V;
    out-projection contracts the local heads.

All matmuls bf16 (fp32 PSUM accumulation); softmax statistics fp32.
"""

from contextlib import ExitStack

import ml_dtypes
import numpy as np

B, T, D = 2, 2048, 2048
H, HD = 16, 128
HHD = HD // 2
HL = 4  # heads per core
NCORES = 8
SCALE = 1.0 / float(np.sqrt(np.float32(HHD)))

TB = 512  # t-superblock (q-block rows, AV free dim)
NTB = T // TB  # 4
DC = 128  # contraction chunk (partition dim)
NDC = D // DC  # 16
NQT = TB // 128  # q-tiles (128 rows) per superblock
SPAN = 1024  # max score/exp span width (2 PSUM banks)

_CACHE = {}


def _build():
    import concourse.mybir as mybir
    from concourse.bacc import Bacc
    from concourse.tile import TileContext

    f32 = mybir.dt.float32
    bf16 = mybir.dt.bfloat16
    Alu = mybir.AluOpType
    Act = mybir.ActivationFunctionType

    nc = Bacc("TRN2", num_devices=NCORES)
    # host-pre-tiled inputs: [128, 16*512] with column block d = rows
    # [128d, 128(d+1)) of the logical [2048, 512] matrix
    wq = nc.declare_dram_parameter("wq", [128, NDC * TB], bf16, isOutput=False)
    wk = nc.declare_dram_parameter("wk", [128, NDC * TB], bf16, isOutput=False)
    wv = nc.declare_dram_parameter("wv", [128, NDC * TB], bf16, isOutput=False)
    won = nc.declare_dram_parameter("won", [128, HL * D], bf16, isOutput=False)
    xts = [
        nc.declare_dram_parameter(f"xt{tb}", [128, NDC * TB], bf16, isOutput=False)
        for tb in range(NTB)
    ]
    lamn = nc.declare_dram_parameter("lamn", [128, HL], f32, isOutput=False)
    msk = nc.declare_dram_parameter("msk", [128, 128], bf16, isOutput=False)
    idn = nc.declare_dram_parameter("idn", [128, 128], bf16, isOutput=False)
    out = nc.declare_dram_parameter("out", [T, D], f32, isOutput=True)

    with TileContext(nc) as tc, ExitStack() as top:
        const = top.enter_context(tc.tile_pool(name="const", bufs=1))
        lam_sb = const.tile([128, HL], f32, tag="lam", name="lam")
        msk_sb = const.tile([128, 128], bf16, tag="msk", name="msk")
        idn_sb = const.tile([128, 128], bf16, tag="idn", name="idn")
        nc.gpsimd.dma_start(out=lam_sb[:], in_=lamn[:])
        nc.gpsimd.dma_start(out=msk_sb[:], in_=msk[:])
        nc.gpsimd.dma_start(out=idn_sb[:], in_=idn[:])

        resid = top.enter_context(tc.tile_pool(name="resid", bufs=1))
        qt_sb = [resid.tile([128, T], bf16, tag=f"qt{h}", name=f"qt{h}") for h in range(HL)]
        kt_sb = [resid.tile([128, T], bf16, tag=f"kt{h}", name=f"kt{h}") for h in range(HL)]
        v_sb = [resid.tile([128, HL * HD], bf16, tag=f"v{s}", name=f"v{s}") for s in range(T // 128)]
        ot_sb = [resid.tile([128, T], bf16, tag=f"ot{h}", name=f"ot{h}") for h in range(HL)]

        # weights: one big tile each, one DMA each (sync queue, in
        # first-needed order); x^T t-blocks on the scalar queue.
        wpool = top.enter_context(tc.tile_pool(name="wpool", bufs=1))
        wq_sb = wpool.tile([128, NDC * TB], bf16, tag="wq", name="wq")
        wk_sb = wpool.tile([128, NDC * TB], bf16, tag="wk", name="wk")
        wv_sb = wpool.tile([128, NDC * TB], bf16, tag="wv", name="wv")
        wo_sb = wpool.tile([128, HL * D], bf16, tag="wo", name="wo")
        nc.sync.dma_start(out=wq_sb[:], in_=wq[:])
        xpool = top.enter_context(tc.tile_pool(name="xpool", bufs=2))
        xt_tiles = {}

        def load_xt(tb):
            t = xpool.tile([128, NDC * TB], bf16, tag="xt", name="xt")
            nc.scalar.dma_start(out=t[:], in_=xts[tb][:])
            xt_tiles[tb] = t

        load_xt(0)
        nc.sync.dma_start(out=wk_sb[:], in_=wk[:])
        load_xt(1)
        nc.sync.dma_start(out=wv_sb[:], in_=wv[:])
        nc.sync.dma_start(out=wo_sb[:], in_=won[:])

        def xc(tb, d):
            return xt_tiles[tb][:, d * TB : (d + 1) * TB]

        # ---- projection piece emitters (yield small chunks of PE work) ----
        def gen_proj_qk(tb, h, which, psp):
            """Q or K projection for (t-block tb, head h): 16 acc matmuls."""
            w_sb, dst = (wq_sb, qt_sb) if which == 0 else (wk_sb, kt_sb)
            ps = psp.tile([128, TB], f32, tag="prj", name="prj")
            for d in range(NDC):
                nc.tensor.matmul(
                    ps[:],
                    lhsT=w_sb[:, d * TB + h * HD : d * TB + (h + 1) * HD],
                    rhs=xc(tb, d),
                    start=(d == 0),
                    stop=(d == NDC - 1),
                )
                if d % 4 == 3 and d != NDC - 1:
                    yield
            nc.vector.tensor_copy(dst[h][:, tb * TB : (tb + 1) * TB], ps[:])
            yield

        def gen_proj_v(tb, tt, psp):
            """V projection for 128-row t-chunk tt of t-block tb."""
            ps = psp.tile([128, HL * HD], f32, tag="prj", name="prj")
            ttsl = slice(tt * 128, (tt + 1) * 128)
            for d in range(NDC):
                nc.tensor.matmul(
                    ps[:],
                    lhsT=xc(tb, d)[:, ttsl],
                    rhs=wv_sb[:, d * TB : (d + 1) * TB],
                    start=(d == 0),
                    stop=(d == NDC - 1),
                )
                if d % 4 == 3 and d != NDC - 1:
                    yield
            nc.vector.tensor_copy(v_sb[tb * NQT + tt][:], ps[:])
            yield

        def gen_proj_tb(tb, psp, load_next=None):
            for h in range(HL):
                yield from gen_proj_qk(tb, h, 0, psp)
            if load_next is not None:
                load_xt(load_next)
            for h in range(HL):
                yield from gen_proj_qk(tb, h, 1, psp)
            for tt in range(NQT):
                yield from gen_proj_v(tb, tt, psp)

        def gen_outproj_qsb(qsb, psp, ospool):
            """Out-projection for q-superblock qsb (all 4 local heads)."""
            for tt in range(NQT):
                t0 = qsb * TB + tt * 128
                for dch2 in range(2):
                    ost = ospool.tile([128, 1024], f32, tag="ost", name="ost")
                    for dd in range(2):
                        d0 = dch2 * 1024 + dd * 512
                        po = psp.tile([128, 512], f32, tag="po", name="po")
                        for h in range(HL):
                            nc.tensor.matmul(
                                po[:],
                                lhsT=ot_sb[h][:, t0 : t0 + 128],
                                rhs=wo_sb[:, h * D + d0 : h * D + d0 + 512],
                                start=(h == 0),
                                stop=(h == HL - 1),
                            )
                        nc.vector.tensor_copy(ost[:, dd * 512 : (dd + 1) * 512], po[:])
                        yield
                    nc.sync.dma_start(
                        out=out[t0 : t0 + 128, dch2 * 1024 : (dch2 + 1) * 1024],
                        in_=ost[:],
                    )

        # ---- phase A: project t-block 0 with a full 8-bank pipeline ----
        with ExitStack() as ph1:
            pps = ph1.enter_context(tc.tile_pool(name="pps", bufs=4, space="PSUM"))
            for _ in gen_proj_tb(0, pps, load_next=1):
                pass

        # ---------------- attention + interleaved fillers ----------------
        with ExitStack() as ph2:
            scps = ph2.enter_context(tc.tile_pool(name="scps", bufs=2, space="PSUM"))
            atps = ph2.enter_context(tc.tile_pool(name="atps", bufs=1, space="PSUM"))
            accps = ph2.enter_context(tc.tile_pool(name="accps", bufs=1, space="PSUM"))
            prjps = ph2.enter_context(tc.tile_pool(name="prjps", bufs=1, space="PSUM"))
            epool = ph2.enter_context(tc.tile_pool(name="epool", bufs=4))
            dpool = ph2.enter_context(tc.tile_pool(name="dpool", bufs=8))
            apool = ph2.enter_context(tc.tile_pool(name="apool", bufs=4))
            spool = ph2.enter_context(tc.tile_pool(name="spool", bufs=3))
            ospool = ph2.enter_context(tc.tile_pool(name="ospool", bufs=2))

            aT_bank = atps.tile([128, 1024], bf16, tag="aT", name="aT")

            filler = [None]

            def pump(n):
                if filler[0] is None:
                    return
                for _ in range(n):
                    try:
                        next(filler[0])
                    except StopIteration:
                        filler[0] = None
                        return

            def chain(*gens):
                for g in gens:
                    yield from g

            def emit_attn_qsb(qsb, rate):
                for h in range(HL):
                    q1 = qt_sb[h][0:64, :]
                    q2 = qt_sb[h][64:128, :]
                    k1 = kt_sb[h][0:64, :]
                    k2 = kt_sb[h][64:128, :]
                    dn_tiles = []  # per qt: list of dn span tiles
                    for qt in range(NQT):
                        tq0 = qsb * TB + qt * 128
                        qsl = slice(tq0, tq0 + 128)
                        S = tq0 + 128  # causal row limit
                        nsp = (S + SPAN - 1) // SPAN
                        pt = spool.tile([128, 4], f32, tag="pt", name="pt")
                        espans = []
                        for sp in range(nsp):
                            c0 = sp * SPAN
                            w = min(SPAN, S - c0)
                            if w <= 512:
                                tile = scps.tile([128, 1024], f32, tag="ps", name="ps")
                                ps1 = tile[:, 0:w]
                                ps2 = tile[:, 512 : 512 + w]
                            else:
                                t1 = scps.tile([128, 1024], f32, tag="ps", name="ps")
                                t2 = scps.tile([128, 1024], f32, tag="ps", name="ps")
                                ps1 = t1[:, 0:w]
                                ps2 = t2[:, 0:w]
                            is_last = sp == nsp - 1
                            for qh, ps in ((q1, ps1), (q2, ps2)):
                                for j in range(0, w, 512):
                                    jw = min(512, w - j)
                                    nc.tensor.matmul(
                                        ps[:, j : j + jw],
                                        lhsT=qh[:, qsl],
                                        rhs=(k1 if qh is q1 else k2)[
                                            :, c0 + j : c0 + j + jw
                                        ],
                                        start=True,
                                        stop=not (is_last and j + jw == w),
                                    )
                                if is_last:
                                    # additive causal mask on the diagonal
                                    # 128 cols via PE: ps += I.T @ msk
                                    dw = w - 128
                                    nc.tensor.matmul(
                                        ps[:, dw : dw + 128],
                                        lhsT=idn_sb[:],
                                        rhs=msk_sb[:],
                                        start=False,
                                        stop=True,
                                        skip_group_check=True,
                                    )
                                pump(rate)
                            e12 = epool.tile([128, 2048], bf16, tag="e", name="e")
                            e1 = e12[:, 0:1024]
                            e2 = e12[:, 1024:2048]
                            nc.scalar.activation(
                                e1[:, :w], ps1[:], Act.Exp, scale=SCALE,
                                accum_out=pt[:, 2 * sp : 2 * sp + 1],
                            )
                            nc.scalar.activation(
                                e2[:, :w], ps2[:], Act.Exp, scale=SCALE,
                                accum_out=pt[:, 2 * sp + 1 : 2 * sp + 2],
                            )
                            espans.append((e1, e2, w))
                            pump(rate)

                        # softmax scalars: r1 = 1/sum1, sv = -lam/sum2
                        rr = spool.tile([128, 2], f32, tag="rr", name="rr")
                        sv = spool.tile([128, 1], f32, tag="sv", name="sv")
                        if nsp > 1:
                            ss = spool.tile([128, 2], f32, tag="ss", name="ss")
                            nc.vector.tensor_tensor(
                                ss[:], pt[:, 0:2], pt[:, 2:4], Alu.add
                            )
                            sums = ss
                        else:
                            sums = pt
                        nc.vector.reciprocal(rr[:], sums[:, 0:2])
                        nc.vector.tensor_scalar(
                            sv[:], rr[:, 1:2], lam_sb[:, h : h + 1], None, Alu.mult
                        )
                        dchunks = []
                        for e1, e2, w in espans:
                            # e1 <- e1/sum1 (in place), dn = e1/sum1 - lam*e2/sum2
                            nc.vector.tensor_scalar(
                                e1[:, :w], e1[:, :w], rr[:, 0:1], None, Alu.mult
                            )
                            dn = dpool.tile([128, 1024], bf16, tag="dn", name="dn")
                            nc.vector.scalar_tensor_tensor(
                                dn[:, :w], e2[:, :w], sv[:], e1[:, :w],
                                Alu.mult, Alu.add,
                            )
                            dchunks.append(dn)
                        dn_tiles.append(dchunks)
                        pump(rate)

                    # transposes (scaled by diag(1/sum1)) + attn @ V
                    s_end = (qsb + 1) * TB
                    nsc = s_end // 128
                    av = accps.tile([128, TB], f32, tag="av", name="av")
                    for k in range(nsc):
                        j0 = 0 if k < qsb * NQT else (k - qsb * NQT)
                        half = (k % 2) * 512
                        aT = aT_bank[:, half : half + 512]
                        for qt in range(j0, NQT):
                            sp, off = divmod(k * 128, SPAN)
                            dn = dn_tiles[qt][sp]
                            nc.tensor.transpose(
                                aT[:, qt * 128 : (qt + 1) * 128],
                                dn[:, off : off + 128],
                                idn_sb[:],
                            )
                        aTs = apool.tile([128, TB], bf16, tag="aTs", name="aTs")
                        nc.vector.tensor_copy(aTs[:, j0 * 128 :], aT[:, j0 * 128 :])
                        nc.tensor.matmul(
                            av[:, j0 * 128 :],
                            lhsT=v_sb[k][:, h * HD : (h + 1) * HD],
                            rhs=aTs[:, j0 * 128 :],
                            start=(k == 0),
                            stop=(k == nsc - 1),
                        )
                        pump(rate)
                    nc.vector.tensor_copy(
                        ot_sb[h][:, qsb * TB : (qsb + 1) * TB], av[:]
                    )
                    pump(2 * rate)

            # fillers: remaining projections + out-projections, pumped into
            # the attention stream in small pieces
            filler[0] = gen_proj_tb(1, prjps, load_next=2)
            emit_attn_qsb(0, rate=3)
            pump(10**9)
            filler[0] = chain(
                gen_proj_tb(2, prjps, load_next=3),
                gen_outproj_qsb(0, prjps, ospool),
            )
            emit_attn_qsb(1, rate=4)
            pump(10**9)
            filler[0] = gen_proj_tb(3, prjps)
            emit_attn_qsb(2, rate=2)
            pump(10**9)
            filler[0] = chain(
                gen_outproj_qsb(1, prjps, ospool),
                gen_outproj_qsb(2, prjps, ospool),
            )
            emit_attn_qsb(3, rate=2)
            pump(10**9)
            for _ in gen_outproj_qsb(3, prjps, ospool):
                pass

    nc.finalize()
    return nc


def _get_nc():
    if "nc" not in _CACHE:
        _CACHE["nc"] = _build()
    return _CACHE["nc"]


def _tile_dmajor(a, rows, cols):
    """[N*rows, cols] -> [rows, N*cols] with column block d = rows [d*rows,...)."""
    n = a.shape[0] // rows
    return np.ascontiguousarray(
        a.reshape(n, rows, cols).transpose(1, 0, 2).reshape(rows, n * cols)
    )


def kernel(x, Wq, Wk, Wv, Wo, lambda_init):
    from concourse.bass_utils import run_bass_kernel_spmd

    bf16 = ml_dtypes.bfloat16
    x = np.asarray(x, dtype=np.float32)
    Wq = np.asarray(Wq, dtype=np.float32)
    Wk = np.asarray(Wk, dtype=np.float32)
    Wv = np.asarray(Wv, dtype=np.float32)
    Wo = np.asarray(Wo, dtype=np.float32)
    lam_full = 1.0 / (1.0 + np.exp(-np.asarray(lambda_init, dtype=np.float32)))

    msk = np.triu(np.full((128, 128), -1e30, np.float32), k=1).astype(bf16)
    idn = np.eye(128, dtype=bf16)

    xt_b = [np.ascontiguousarray(x[b].T).astype(bf16) for b in range(B)]
    in_maps = []
    for c in range(NCORES):
        b, g = divmod(c, NCORES // B)  # b = c // 4, g = c % 4
        cols = slice(g * HL * HD, (g + 1) * HL * HD)
        im = {
            "wq": _tile_dmajor(Wq[:, cols].astype(bf16), 128, 512),
            "wk": _tile_dmajor(Wk[:, cols].astype(bf16), 128, 512),
            "wv": _tile_dmajor(Wv[:, cols].astype(bf16), 128, 512),
            "won": _tile_dmajor(Wo[cols, :].astype(bf16), 128, 2048),
            "lamn": np.tile(-lam_full[g * HL : (g + 1) * HL], (128, 1)).astype(
                np.float32
            ),
            "msk": msk,
            "idn": idn,
        }
        for tb in range(NTB):
            im[f"xt{tb}"] = _tile_dmajor(
                np.ascontiguousarray(xt_b[b][:, tb * TB : (tb + 1) * TB]), 128, 512
            )
        in_maps.append(im)

    nc = _get_nc()
    res = run_bass_kernel_spmd(nc, in_maps, core_ids=list(range(NCORES)))
    _CACHE["last_results"] = res  # exec_time_ns etc. when tracing is enabled

    full = np.zeros((B, T, D), np.float32)
    for c in range(NCORES):
        b = c // (NCORES // B)
        full[b] += res.results[c]["out"]
    return full
